# revision 2
# baseline (speedup 1.0000x reference)
"""LlamaAttention (GQA, no mask) on 8 Trainium2 NeuronCores.

Sharding: 8 cores = 2 (batch) x 4 (head groups of 8 heads / 2 KV heads).
Per core (all bf16 compute, fp32 accumulation):
  qT  = (x_b @ wq_g)^T            [512, 2048]   (head dims on partitions)
  kTd = (x_b @ wk_g)^T duplicated [128, 2, 2048]
  v   = x_b @ wv_g (+ ones col)   [2048, 2, 65]
  per head: sT[k,q] = kT^T-style matmul -> exp on ACT -> (v|1)^T @ pT
            gives attn-out^T rows 0:64 and softmax denominator in row 64
  out_partial = attn_out @ wo_g   [2048, 2048] fp32
Host sums the 4 head-group partials per batch.
"""

import numpy as np
import ml_dtypes

S = 2048          # sequence length
D = 2048          # model dim
HD = 64           # head dim
GH = 8            # heads per core
QC = GH * HD      # 512 q cols per core
KVC = 128         # kv cols per core (2 kv heads)
DC = D // 128     # 16 contraction chunks
SC = S // 128     # 16 seq chunks
NB = S // 512     # 4 seq blocks of 512
JB = 2            # q blocks of 1024 for attention
SCALE = HD ** -0.5

_CACHE = {}


def _build():
    import concourse.bass as bass
    import concourse.mybir as mybir
    import concourse.tile as tile
    from concourse import bacc

    f32 = mybir.dt.float32
    bf16 = mybir.dt.bfloat16
    Exp = mybir.ActivationFunctionType.Exp

    nc = bacc.Bacc("TRN2", target_bir_lowering=False, debug=False, num_devices=8)

    xt = nc.dram_tensor("xt", [D, S], bf16, kind="ExternalInput").ap()
    wq = nc.dram_tensor("wq", [D, QC], bf16, kind="ExternalInput").ap()
    wk = nc.dram_tensor("wk", [D, KVC], bf16, kind="ExternalInput").ap()
    wv = nc.dram_tensor("wv", [D, KVC], bf16, kind="ExternalInput").ap()
    wo = nc.dram_tensor("wo", [QC, D], bf16, kind="ExternalInput").ap()
    out = nc.dram_tensor("out", [S, D], f32, kind="ExternalOutput").ap()

    with tile.TileContext(nc) as tc:
        with tc.tile_pool(name="const", bufs=1) as const:
            # resident inputs, partition-chunked layouts
            xt_all = const.tile([128, DC, S], bf16, tag="xt_all")
            nc.sync.dma_start(out=xt_all[:], in_=xt.rearrange("(c p) s -> p c s", p=128))
            wq_all = const.tile([128, DC, QC], bf16, tag="wq_all")
            nc.sync.dma_start(out=wq_all[:], in_=wq.rearrange("(c p) n -> p c n", p=128))
            wk_all = const.tile([128, DC, KVC], bf16, tag="wk_all")
            nc.sync.dma_start(out=wk_all[:], in_=wk.rearrange("(c p) n -> p c n", p=128))
            wv_all = const.tile([128, DC, KVC], bf16, tag="wv_all")
            nc.sync.dma_start(out=wv_all[:], in_=wv.rearrange("(c p) n -> p c n", p=128))
            wo_all = const.tile([128, QC // 128, D], bf16, tag="wo_all")
            nc.sync.dma_start(out=wo_all[:], in_=wo.rearrange("(c p) n -> p c n", p=128))

            # persistent intermediates
            qpair = const.tile([128, 4, S], bf16, tag="qpair")     # q^T, 2 heads per ptile
            ktd = const.tile([128, 2, S], bf16, tag="ktd")         # k^T per kv head, duplicated
            vv = const.tile([128, SC, 130], bf16, tag="vv")        # v (+ones col) per kv head
            at = const.tile([128, 4, S], bf16, tag="at")           # attn out^T

            # ---------------- projections ----------------
            with tc.tile_pool(name="pp", bufs=4, space="PSUM") as pp, \
                 tc.tile_pool(name="ev", bufs=3) as ev:
                # k^T : [128, S]; evict duplicated per kv head
                for nb in range(NB):
                    ps = pp.tile([128, 512], f32, tag="ps")
                    for dc in range(DC):
                        nc.tensor.matmul(ps[:], wk_all[:, dc, :],
                                         xt_all[:, dc, nb * 512:(nb + 1) * 512],
                                         start=(dc == 0), stop=(dc == DC - 1))
                    kt_sb = ev.tile([128, 512], bf16, tag="kt_sb")
                    nc.vector.tensor_copy(kt_sb[:], ps[:])
                    sl = slice(nb * 512, (nb + 1) * 512)
                    # kv0 rows 0:64 -> ktd[:,0], kv1 rows 64:128 -> ktd[:,1] (dup)
                    nc.sync.dma_start(out=ktd[0:64, 0, sl], in_=kt_sb[0:64, :])
                    nc.sync.dma_start(out=ktd[64:128, 0, sl], in_=kt_sb[0:64, :])
                    nc.sync.dma_start(out=ktd[0:64, 1, sl], in_=kt_sb[64:128, :])
                    nc.sync.dma_start(out=ktd[64:128, 1, sl], in_=kt_sb[64:128, :])

                # v : [S, 128] seq-chunked, with ones columns at 64 and 129
                for sc in range(SC):
                    ps = pp.tile([128, 512], f32, tag="ps")
                    for dc in range(DC):
                        nc.tensor.matmul(ps[:, 0:KVC],
                                         xt_all[:, dc, sc * 128:(sc + 1) * 128],
                                         wv_all[:, dc, :],
                                         start=(dc == 0), stop=(dc == DC - 1))
                    nc.vector.tensor_copy(vv[:, sc, 0:64], ps[:, 0:64])
                    nc.vector.tensor_copy(vv[:, sc, 65:129], ps[:, 64:128])
                    nc.vector.memset(vv[:, sc, 64:65], 1.0)
                    nc.vector.memset(vv[:, sc, 129:130], 1.0)

                # q^T : 4 partition tiles of [128, S]
                for qm in range(4):
                    for nb in range(NB):
                        ps = pp.tile([128, 512], f32, tag="ps")
                        for dc in range(DC):
                            nc.tensor.matmul(ps[:],
                                             wq_all[:, dc, qm * 128:(qm + 1) * 128],
                                             xt_all[:, dc, nb * 512:(nb + 1) * 512],
                                             start=(dc == 0), stop=(dc == DC - 1))
                        nc.vector.tensor_copy(qpair[:, qm, nb * 512:(nb + 1) * 512], ps[:])

            # ---------------- attention ----------------
            with tc.tile_pool(name="sps", bufs=2, space="PSUM") as sps, \
                 tc.tile_pool(name="ops", bufs=2, space="PSUM") as ops, \
                 tc.tile_pool(name="ppool", bufs=3) as ppool, \
                 tc.tile_pool(name="dpool", bufs=4) as dpool:
                for qm in range(4):
                    kv = qm // 2
                    for h2 in range(2):
                        hb = 64 * h2          # partition base of this head in qpair/ktd
                        h = 2 * qm + h2       # local head id
                        for jb in range(JB):
                            qsl = slice(jb * 1024, (jb + 1) * 1024)
                            o_ps = ops.tile([65, 1024], f32, tag="o_ps")
                            for kc in range(SC):
                                s_ps = sps.tile([128, 1024], f32, tag="s_ps")
                                for hf in range(2):
                                    nc.tensor.matmul(
                                        s_ps[:, hf * 512:(hf + 1) * 512],
                                        ktd[hb:hb + 64, kv, kc * 128:(kc + 1) * 128],
                                        qpair[hb:hb + 64, qm,
                                              jb * 1024 + hf * 512:jb * 1024 + (hf + 1) * 512],
                                        start=True, stop=True)
                                p_t = ppool.tile([128, 1024], bf16, tag="p_t")
                                nc.scalar.activation(p_t[:], s_ps[:], Exp, scale=SCALE)
                                for hf in range(2):
                                    nc.tensor.matmul(
                                        o_ps[:, hf * 512:(hf + 1) * 512],
                                        vv[:, kc, kv * 65:kv * 65 + 65],
                                        p_t[:, hf * 512:(hf + 1) * 512],
                                        start=(kc == 0), stop=(kc == SC - 1))
                            # normalize: row 64 is the softmax denominator
                            rden = dpool.tile([1, 1024], f32, tag="rden")
                            nc.vector.reciprocal(rden[:], o_ps[64:65, :])
                            rdenb = dpool.tile([64, 1024], f32, tag="rdenb")
                            nc.gpsimd.partition_broadcast(rdenb[:], rden[:])
                            nc.vector.tensor_mul(at[hb:hb + 64, h // 2, qsl],
                                                 o_ps[0:64, :], rdenb[:])

            # ---------------- o_proj ----------------
            with tc.tile_pool(name="pp2", bufs=4, space="PSUM") as pp2, \
                 tc.tile_pool(name="osb", bufs=3) as osb:
                for sm in range(SC):
                    for nb in range(NB):
                        ps = pp2.tile([128, 512], f32, tag="ps2")
                        for cc in range(4):
                            nc.tensor.matmul(ps[:],
                                             at[:, cc, sm * 128:(sm + 1) * 128],
                                             wo_all[:, cc, nb * 512:(nb + 1) * 512],
                                             start=(cc == 0), stop=(cc == 3))
                        o_sb = osb.tile([128, 512], f32, tag="o_sb")
                        nc.vector.tensor_copy(o_sb[:], ps[:])
                        nc.sync.dma_start(
                            out=out[sm * 128:(sm + 1) * 128, nb * 512:(nb + 1) * 512],
                            in_=o_sb[:])

    nc.compile()
    return nc


def _get_nc():
    if "nc" not in _CACHE:
        _CACHE["nc"] = _build()
    return _CACHE["nc"]


def kernel(x, wq, wk, wv, wo):
    from concourse.bass_utils import run_bass_kernel_spmd

    bf16 = ml_dtypes.bfloat16
    nc = _get_nc()

    in_maps = []
    for core in range(8):
        b, g = core // 4, core % 4
        in_maps.append({
            "xt": np.ascontiguousarray(np.asarray(x)[b].T).astype(bf16),
            "wq": np.ascontiguousarray(np.asarray(wq)[:, g * QC:(g + 1) * QC]).astype(bf16),
            "wk": np.ascontiguousarray(np.asarray(wk)[:, g * KVC:(g + 1) * KVC]).astype(bf16),
            "wv": np.ascontiguousarray(np.asarray(wv)[:, g * KVC:(g + 1) * KVC]).astype(bf16),
            "wo": np.ascontiguousarray(np.asarray(wo)[g * QC:(g + 1) * QC, :]).astype(bf16),
        })

    res = run_bass_kernel_spmd(nc, in_maps, core_ids=list(range(8)))
    outs = [res.results[c]["out"] for c in range(8)]
    full = np.empty((2, S, D), np.float32)
    full[0] = outs[0] + outs[1] + outs[2] + outs[3]
    full[1] = outs[4] + outs[5] + outs[6] + outs[7]
    return full


# revision 8
# speedup vs baseline: 10343.5546x; 10343.5546x over previous
"""LlamaAttention (GQA, no mask) on 8 Trainium2 NeuronCores.

Sharding: 8 cores = 2 (batch) x 4 (head groups of 8 heads / 2 KV heads).
Per core (all bf16 compute, fp32 accumulation):
  qT  = (x_b @ wq_g)^T            [512, 2048]   (head dims on partitions)
  kTd = (x_b @ wk_g)^T duplicated [128, 2, 2048]
  v   = x_b @ wv_g (+ ones col)   [2048, 2, 65]
  per head: sT[k,q] = k^T-layout matmul -> exp on ACT -> (v|1)^T @ pT
            gives attn-out^T rows 0:64 and softmax denominator in row 64
  out_partial = attn_out @ wo_g   [2048, 2048] fp32
Host sums the 4 head-group partials per batch.
Phases are fused per 1024-wide q block so o_proj and q-proj matmuls fill
the PE slack of the ACT-bound attention inner loop.
"""

import numpy as np
import ml_dtypes

S = 2048          # sequence length
D = 2048          # model dim
HD = 64           # head dim
GH = 8            # heads per core
QC = GH * HD      # 512 q cols per core
KVC = 128         # kv cols per core (2 kv heads)
DC = D // 128     # 16 contraction chunks
SC = S // 128     # 16 seq chunks
SCALE = HD ** -0.5

_CACHE = {}


def _build():
    import concourse.bass as bass
    import concourse.mybir as mybir
    import concourse.tile as tile
    from concourse import bacc

    f32 = mybir.dt.float32
    bf16 = mybir.dt.bfloat16
    Exp = mybir.ActivationFunctionType.Exp

    nc = bacc.Bacc("TRN2", target_bir_lowering=False, debug=False, num_devices=8)

    xt = nc.dram_tensor("xt", [D, S], bf16, kind="ExternalInput").ap()
    wq = nc.dram_tensor("wq", [D, QC], bf16, kind="ExternalInput").ap()
    wk = nc.dram_tensor("wk", [D, KVC], bf16, kind="ExternalInput").ap()
    wv = nc.dram_tensor("wv", [D, KVC], bf16, kind="ExternalInput").ap()
    wo = nc.dram_tensor("wo", [QC, D], bf16, kind="ExternalInput").ap()
    out = nc.dram_tensor("out", [S, D], f32, kind="ExternalOutput").ap()

    with tile.TileContext(nc) as tc:
        with tc.tile_pool(name="const", bufs=1) as const, \
             tc.tile_pool(name="mm", bufs=2, space="PSUM") as mm, \
             tc.tile_pool(name="ops", bufs=2, space="PSUM") as ops, \
             tc.tile_pool(name="ev", bufs=3) as ev, \
             tc.tile_pool(name="ppool", bufs=3) as ppool, \
             tc.tile_pool(name="dpool", bufs=4) as dpool:

            # resident inputs, partition-chunked layouts (small weights first,
            # xt per-chunk so the k/v projections can start early)
            wk_all = const.tile([128, DC, KVC], bf16, tag="wk_all")
            nc.sync.dma_start(out=wk_all[:], in_=wk.rearrange("(c p) n -> p c n", p=128))
            wv_all = const.tile([128, DC, KVC], bf16, tag="wv_all")
            nc.sync.dma_start(out=wv_all[:], in_=wv.rearrange("(c p) n -> p c n", p=128))
            xt_all = const.tile([128, DC, S], bf16, tag="xt_all")
            xt_re = xt.rearrange("(c p) s -> p c s", p=128)
            for dc in range(DC):
                nc.sync.dma_start(out=xt_all[:, dc, :], in_=xt_re[:, dc, :])
            wq_all = const.tile([128, DC, QC], bf16, tag="wq_all")
            nc.sync.dma_start(out=wq_all[:], in_=wq.rearrange("(c p) n -> p c n", p=128))
            wo_all = const.tile([128, QC // 128, D], bf16, tag="wo_all")
            nc.sync.dma_start(out=wo_all[:], in_=wo.rearrange("(c p) n -> p c n", p=128))

            # persistent intermediates
            qpair = const.tile([128, 4, S], bf16, tag="qpair")     # q^T
            ktd = const.tile([128, 2, S], bf16, tag="ktd")         # k^T dup per kv head
            vv = const.tile([128, SC, 130], bf16, tag="vv")        # v (+ones cols)
            at = const.tile([128, 4, S], bf16, tag="at")           # attn out^T


            def mmacc(out_t, lhsT, rhs, width, start, stop):
                # moving-operand ISA limit is 512: split wide matmuls
                for o in range(0, width, 512):
                    nc.tensor.matmul(out_t[:, o:o + 512], lhsT,
                                     rhs[:, o:o + 512], start=start, stop=stop)
            # ---------------- k/v projections ----------------
            for nb in range(2):
                ps = mm.tile([128, 1024], f32, tag="mm_ps")
                for dc in range(DC):
                    mmacc(ps, wk_all[:, dc, :],
                          xt_all[:, dc, nb * 1024:(nb + 1) * 1024], 1024,
                          (dc == 0), (dc == DC - 1))
                kt_sb = ev.tile([128, 1024], bf16, tag="kt_sb")
                nc.vector.tensor_copy(kt_sb[:], ps[:])
                sl = slice(nb * 1024, (nb + 1) * 1024)
                nc.sync.dma_start(out=ktd[0:64, 0, sl], in_=kt_sb[0:64, :])
                nc.sync.dma_start(out=ktd[64:128, 0, sl], in_=kt_sb[0:64, :])
                nc.sync.dma_start(out=ktd[0:64, 1, sl], in_=kt_sb[64:128, :])
                nc.sync.dma_start(out=ktd[64:128, 1, sl], in_=kt_sb[64:128, :])

            for sc in range(SC):
                ps = mm.tile([128, 1024], f32, tag="mm_ps")
                for dc in range(DC):
                    nc.tensor.matmul(ps[:, 0:KVC],
                                     xt_all[:, dc, sc * 128:(sc + 1) * 128],
                                     wv_all[:, dc, :],
                                     start=(dc == 0), stop=(dc == DC - 1))
                nc.vector.tensor_copy(vv[:, sc, 0:64], ps[:, 0:64])
                nc.vector.tensor_copy(vv[:, sc, 65:129], ps[:, 64:128])
                nc.vector.memset(vv[:, sc, 64:65], 1.0)
                nc.vector.memset(vv[:, sc, 129:130], 1.0)

            # ------------- fused q-proj + attention + o_proj, per q block -------------
            def oproj(sm):
                for nb in range(2):
                    ps = mm.tile([128, 1024], f32, tag="mm_ps")
                    for cc in range(4):
                        mmacc(ps, at[:, cc, sm * 128:(sm + 1) * 128],
                              wo_all[:, cc, nb * 1024:(nb + 1) * 1024], 1024,
                              (cc == 0), (cc == 3))
                    o_sb = ev.tile([128, 1024], f32, tag="o_sb")
                    nc.vector.tensor_copy(o_sb[:], ps[:])
                    nc.sync.dma_start(
                        out=out[sm * 128:(sm + 1) * 128,
                                nb * 1024:(nb + 1) * 1024],
                        in_=o_sb[:])

            pending = []        # seq tiles whose o_proj is ready to interleave
            for jb in range(2):
                qsl = slice(jb * 1024, (jb + 1) * 1024)
                for qm in range(4):
                    kv = qm // 2
                    # q^T projection for this (qm, jb) slice
                    ps = mm.tile([128, 1024], f32, tag="mm_ps")
                    for dc in range(DC):
                        mmacc(ps, wq_all[:, dc, qm * 128:(qm + 1) * 128],
                              xt_all[:, dc, qsl], 1024,
                              (dc == 0), (dc == DC - 1))
                    nc.vector.tensor_copy(qpair[:, qm, qsl], ps[:])

                    # both heads of the pair per k-chunk: their scores matmuls
                    # sit in disjoint PE row-groups (partition bases 0 / 64)
                    # and run concurrently in the array
                    o_A = ops.tile([65, 1024], f32, tag="o_ps")
                    o_B = ops.tile([65, 1024], f32, tag="o_ps")
                    for kc in range(SC):
                        ksl = slice(kc * 128, (kc + 1) * 128)
                        s_A = mm.tile([128, 1024], f32, tag="mm_ps")
                        s_B = mm.tile([128, 1024], f32, tag="mm_ps")
                        mmacc(s_A, ktd[0:64, kv, ksl],
                              qpair[0:64, qm, qsl], 1024, True, True)
                        mmacc(s_B, ktd[64:128, kv, ksl],
                              qpair[64:128, qm, qsl], 1024, True, True)
                        p_A = ppool.tile([128, 1024], bf16, tag="p_t")
                        nc.scalar.activation(p_A[:], s_A[:], Exp, scale=SCALE)
                        p_B = ppool.tile([128, 1024], bf16, tag="p_t")
                        nc.scalar.activation(p_B[:], s_B[:], Exp, scale=SCALE)
                        mmacc(o_A, vv[:, kc, kv * 65:kv * 65 + 65],
                              p_A, 1024, (kc == 0), (kc == SC - 1))
                        mmacc(o_B, vv[:, kc, kv * 65:kv * 65 + 65],
                              p_B, 1024, (kc == 0), (kc == SC - 1))
                    for h2, o_ps in ((0, o_A), (1, o_B)):
                        hb = 64 * h2
                        h = 2 * qm + h2
                        # normalize by the softmax denominator (row 64)
                        rden = dpool.tile([1, 1024], f32, tag="rden")
                        nc.vector.reciprocal(rden[:], o_ps[64:65, :])
                        rdenb = dpool.tile([64, 1024], f32, tag="rdenb")
                        nc.gpsimd.partition_broadcast(rdenb[:], rden[:])
                        nc.vector.tensor_mul(at[hb:hb + 64, h // 2, qsl],
                                             o_ps[0:64, :], rdenb[:])
                        # interleave a pending o_proj seq tile (previous block)
                        if pending:
                            oproj(pending.pop(0))
                pending.extend(range(jb * 8, (jb + 1) * 8))
            for sm in pending:
                oproj(sm)

    nc.compile()
    return nc


def _get_nc():
    if "nc" not in _CACHE:
        _CACHE["nc"] = _build()
    return _CACHE["nc"]


def kernel(x, wq, wk, wv, wo):
    from concourse.bass_utils import run_bass_kernel_spmd

    bf16 = ml_dtypes.bfloat16
    nc = _get_nc()

    in_maps = []
    for core in range(8):
        b, g = core // 4, core % 4
        in_maps.append({
            "xt": np.ascontiguousarray(np.asarray(x)[b].T).astype(bf16),
            "wq": np.ascontiguousarray(np.asarray(wq)[:, g * QC:(g + 1) * QC]).astype(bf16),
            "wk": np.ascontiguousarray(np.asarray(wk)[:, g * KVC:(g + 1) * KVC]).astype(bf16),
            "wv": np.ascontiguousarray(np.asarray(wv)[:, g * KVC:(g + 1) * KVC]).astype(bf16),
            "wo": np.ascontiguousarray(np.asarray(wo)[g * QC:(g + 1) * QC, :]).astype(bf16),
        })

    res = run_bass_kernel_spmd(nc, in_maps, core_ids=list(range(8)))
    outs = [res.results[c]["out"] for c in range(8)]
    full = np.empty((2, S, D), np.float32)
    full[0] = outs[0] + outs[1] + outs[2] + outs[3]
    full[1] = outs[4] + outs[5] + outs[6] + outs[7]
    return full
